# revision 1
# baseline (speedup 1.0000x reference)
"""DiceLoss Trainium2 kernel (sorted-pixel design).

Math: for preds [B,C,H,W] (logits), integer targets [B,H,W]:
  P = softmax over C;  S_c = sum_n P_nc;  D_c = sum_{n: t_n=c} P_{n,t_n}
  N_c = count(target==c); then TP/FP/FN -> alpha -> dice -> loss (host, 32-vec).

Device layout (per core = one batch): 128 SBUF partitions = 4 pixel
groups x 32 classes; free dim = pixels. The HOST SORTS pixels by target
class within each group, so "target == c" becomes a contiguous index
range per (group, class) row. Per tile [128, K]:
  ACT : E = exp(X) -> bf16
  PE  : Zrep = blockdiag(1s) @ E  (per-pixel sum over classes, PSUM)
  R   : 1/Z -> bf16, via custom-DVE reciprocal_approx (some tiles) or
        ACT Ln -> Exp(-L) (other tiles) to balance engine load
  DVE : DICE_RANGE_REDUCE(E, R, [0,K))      -> S partials [128,1]
  DVE : DICE_RANGE_REDUCE(E, R, [lo,hi))    -> D partials [128,1]
Host reduces the [128, n_tiles] partials and finishes the algebra.
"""

import numpy as np
import ml_dtypes

import concourse.bass as bass
import concourse.bacc as bacc
import concourse.mybir as mybir
from concourse.tile import TileContext
from concourse.bass_utils import run_bass_kernel_spmd

# ---- custom DVE op: range-masked multiply-reduce (self-contained) ---------
# accum_out[p] = sum_k (in0[p,k]*in1[p,k]) * (s0[p] <= k < s1[p])


def _make_dice_range_op():
    import re
    import concourse.dve_ops as dve_ops
    from concourse.dve_ops import DveOp
    from concourse.dve_spec import Spec, Src0, Src1, C0, C1, Idx, Zero
    from operator import add

    name = "DICE_RANGE_REDUCE"
    if name in dve_ops._SUB_OPCODE_FOR_NAME:
        for op in dve_ops.OPS:
            if op.name == name:
                return op

    def _ref(in0, in1, s0, s1, imm2):
        n = in0.shape[-1]
        idx = np.arange(n, dtype=np.float32)
        s0 = np.asarray(s0, dtype=np.float32).reshape(-1, 1)
        s1 = np.asarray(s1, dtype=np.float32).reshape(-1, 1)
        mask = ((idx >= s0) & (idx < s1)).astype(np.float32)
        b = (in0.astype(np.float32) * in1.astype(np.float32) * mask).astype(np.float32)
        return b, b.reshape(b.shape[0], -1).sum(axis=-1, keepdims=True)

    spec = Spec(
        body=(Src0 * Src1) * ((Idx >= C0) & (Idx < C1)),
        accum=add,
        accum_init=Zero,
        reference=_ref,
    )
    row = dve_ops._CUSTOM_DVE_ROW_BASE + len(dve_ops.OPS)
    assert row < 0x20
    op = DveOp(name, spec, subdim=False, uops_sha={})
    dve_ops.OPS.append(op)
    dve_ops.CUSTOM_DVE_SPECS[name] = spec
    dve_ops._SUB_OPCODE_FOR_NAME[name] = row
    for ver in ("v3", "v4"):
        try:
            op.compile(ver)
        except ValueError as e:
            m = re.search(r'uops_sha\["%s"\]="([0-9a-f]+)"' % ver, str(e))
            if not m:
                raise
            op.uops_sha[ver] = m.group(1)
            dve_ops._COMPILE_CACHE.pop((name, ver), None)
        op.compile(ver)
    return op


DICE_RANGE_REDUCE = _make_dice_range_op()

# Problem shapes (hardcoded per contract).
B, C, H, W = 8, 32, 512, 512
HW = H * W            # 262144 pixels per batch
G = 4                 # pixel groups sharing the 128 partitions
GPIX = HW // G        # 65536 pixels per group
EPS = 1e-8
SMOOTH = 1e-5
NCORES = 8

F32 = mybir.dt.float32
BF16 = mybir.dt.bfloat16
BF = ml_dtypes.bfloat16


def _patch_act_tables():
    """Order activation tables so the set containing BOTH Exp and Ln is
    preferred - otherwise the table-load pass thrashes between the exp-only
    and ln-only sets (one ~2.7us reload per tile)."""
    import concourse.bacc as _bacc
    if getattr(_bacc, "_dice_tables_patched", False):
        return
    orig = _bacc.get_activation_tables

    def filtered(arch):
        # PRESERVE dict order (set ids are positional indexes into
        # act_info.json) - only hide Exp/Ln from the single-function sets so
        # the pass must pick the combined one.
        tabs = dict(orig(arch))
        if "natural_log_exp_and_others" not in tabs:
            return tabs
        import concourse.mybir as mb
        out = {}
        for name, funcs in tabs.items():
            if name != "natural_log_exp_and_others":
                funcs = {f for f in funcs
                         if f not in (mb.ActivationFunctionType.Exp,
                                      mb.ActivationFunctionType.Ln)}
            out[name] = funcs
        return out

    _bacc.get_activation_tables = filtered
    _bacc._dice_tables_patched = True


def build_nc(gpix=GPIX, k=2048, n_dve_recip=7, process_nt=None, psum_bufs=2):
    """One-core SPMD program. Every n-th tile computes 1/Z on the DVE
    (reciprocal_approx); the rest use the ACT Ln->Exp chain, balancing the
    two engines. process_nt: only emit compute for the first N tiles
    (benchmarking aid - transfers stay identical)."""
    from concourse.dve_ops import RECIP_APPROX_FAST_CONSTS, RECIPROCAL_APPROX_FAST

    _patch_act_tables()
    nt = gpix // k
    nc = bacc.Bacc("TRN2", target_bir_lowering=False)
    x = nc.declare_dram_parameter("x", [128, gpix], F32, isOutput=False)
    lo = nc.declare_dram_parameter("lo", [128, nt], F32, isOutput=False)
    hi = nc.declare_dram_parameter("hi", [128, nt], F32, isOutput=False)
    w1 = nc.declare_dram_parameter("w1", [128, 128], BF16, isOutput=False)
    s_out = nc.declare_dram_parameter("s_out", [128, nt], F32, isOutput=True)
    d_out = nc.declare_dram_parameter("d_out", [128, nt], F32, isOutput=True)

    cst = RECIP_APPROX_FAST_CONSTS
    MMF = 512  # columns per matmul instruction
    # spread the DVE-recip tiles evenly (edge-packing measured worse: 201 vs 181us)
    dve_tiles = set(round(i * (nt - 1) / max(n_dve_recip - 1, 1)) for i in range(n_dve_recip)) if n_dve_recip else set()

    with TileContext(nc) as tc:
        big = k > 2048
        with (
            tc.tile_pool(name="const", bufs=1) as constp,
            tc.tile_pool(name="xin", bufs=3 if big else 6) as xp,
            tc.tile_pool(name="ework", bufs=4 if big else 8) as ep,
            tc.tile_pool(name="lwork", bufs=2 if big else 3) as lp,
            tc.tile_pool(name="rwork", bufs=2 if big else 4) as rp,
            tc.tile_pool(name="junk", bufs=4) as jp,
            tc.tile_pool(name="acc", bufs=1) as accp,
            tc.tile_pool(name="ps1", bufs=psum_bufs, space="PSUM") as ps1,
        ):
            w1_t = constp.tile([128, 128], BF16)
            nc.sync.dma_start(out=w1_t[:], in_=w1[:])
            lo_t = constp.tile([128, nt], F32)
            nc.sync.dma_start(out=lo_t[:], in_=lo[:])
            hi_t = constp.tile([128, nt], F32)
            nc.sync.dma_start(out=hi_t[:], in_=hi[:])
            s_acc = accp.tile([128, nt], F32)
            d_acc = accp.tile([128, nt], F32)

            for t in range(nt if process_nt is None else process_nt):
                xt = xp.tile([128, k], F32)
                nc.sync.dma_start(out=xt[:], in_=x[:, t * k:(t + 1) * k])
                et = ep.tile([128, k], BF16)
                nc.scalar.activation(et[:], xt[:], mybir.ActivationFunctionType.Exp)
                z_ps = ps1.tile([128, k], F32)
                for m0 in range(0, k, MMF):
                    nc.tensor.matmul(
                        z_ps[:, m0:m0 + MMF], w1_t[:], et[:, m0:m0 + MMF],
                        start=True, stop=True,
                    )
                rt = rp.tile([128, k], BF16, tag="r")
                if t in dve_tiles:
                    nc.vector._custom_dve(
                        RECIPROCAL_APPROX_FAST, out=rt[:], in0=z_ps[:],
                        s0=cst["s0"], s1=cst["s1"], imm2=cst["imm2"],
                    )
                else:
                    lt = lp.tile([128, k], F32, tag="l")
                    nc.scalar.activation(lt[:], z_ps[:], mybir.ActivationFunctionType.Ln)
                    nc.scalar.activation(rt[:], lt[:], mybir.ActivationFunctionType.Exp,
                                         scale=-1.0)
                j1 = jp.tile([128, 1], BF16, tag="j1")
                nc.vector._custom_dve(
                    DICE_RANGE_REDUCE, out=j1[:].broadcast_to((128, k)),
                    in0=et[:], in1=rt[:],
                    s0=0.0, s1=float(k), accum_out=s_acc[:, t:t + 1],
                )
                j2 = jp.tile([128, 1], BF16, tag="j2")
                nc.vector._custom_dve(
                    DICE_RANGE_REDUCE, out=j2[:].broadcast_to((128, k)),
                    in0=et[:], in1=rt[:],
                    s0=lo_t[:, t:t + 1], s1=hi_t[:, t:t + 1],
                    accum_out=d_acc[:, t:t + 1],
                )
            nc.sync.dma_start(out=s_out[:], in_=s_acc[:])
            nc.sync.dma_start(out=d_out[:], in_=d_acc[:])
    nc.finalize()
    return nc


def host_w1():
    w1 = np.zeros((128, 128), dtype=BF)
    for g in range(G):
        w1[g * 32:(g + 1) * 32, g * 32:(g + 1) * 32] = BF(1.0)
    return w1


def host_prep(preds_b, targets_b, gpix=GPIX, k=2048):
    """Sort pixels by target within each group; build x [128, gpix] plus
    per-tile class-range bounds lo/hi [128, nt]."""
    nt = gpix // k
    xc = preds_b.reshape(C, G, gpix)
    tg = targets_b.reshape(G, gpix)
    x = np.empty((G * C, gpix), dtype=np.float32)
    lo = np.zeros((G * C, nt), dtype=np.float32)
    hi = np.zeros((G * C, nt), dtype=np.float32)
    for g in range(G):
        perm = np.argsort(tg[g], kind="stable")
        x[g * C:(g + 1) * C, :] = xc[:, g, :][:, perm]
        counts = np.bincount(tg[g].astype(np.int64), minlength=C)
        ends = np.cumsum(counts)
        starts = ends - counts
        for t in range(nt):
            lo[g * C:(g + 1) * C, t] = np.clip(starts - t * k, 0, k)
            hi[g * C:(g + 1) * C, t] = np.clip(ends - t * k, 0, k)
    return x, lo, hi


def finish_loss(S, D, Ncnt, npix_total):
    """Host-side 32-vector algebra, mirrors the reference exactly."""
    S = S.astype(np.float64)
    D = D.astype(np.float64)
    Ncnt = Ncnt.astype(np.float64)
    TP = EPS * S + (1.0 - EPS) * D
    FP = S - TP
    FN = (EPS * npix_total + (1.0 - EPS) * Ncnt) - TP
    alpha = np.clip(FP / (FP + FN + SMOOTH), 0.2, 0.8)
    beta = 1.0 - alpha
    den = TP + alpha * FP + beta * FN
    dice = TP / (den + SMOOTH)
    loss = np.sum(1.0 - dice) / C
    return np.float32(loss)


_NC_CACHE = {}


def _get_nc():
    if "nc" not in _NC_CACHE:
        _NC_CACHE["nc"] = build_nc()
    return _NC_CACHE["nc"]


def kernel(preds, targets):
    preds = np.asarray(preds, dtype=np.float32)
    targets = np.asarray(targets)
    nc = _get_nc()
    w1 = host_w1()
    in_maps = []
    for b in range(NCORES):
        x, lo, hi = host_prep(preds[b].reshape(C, HW), targets[b].reshape(-1))
        in_maps.append({"x": x, "lo": lo, "hi": hi, "w1": w1})
    res = run_bass_kernel_spmd(nc, in_maps, list(range(NCORES))).results
    S = np.zeros(C, dtype=np.float64)
    D = np.zeros(C, dtype=np.float64)
    for b in range(NCORES):
        so = np.asarray(res[b]["s_out"], dtype=np.float64)  # [128, nt]
        do = np.asarray(res[b]["d_out"], dtype=np.float64)
        S += so.sum(axis=1).reshape(G, C).sum(axis=0)
        D += do.sum(axis=1).reshape(G, C).sum(axis=0)
    Ncnt = np.bincount(targets.reshape(-1).astype(np.int64), minlength=C).astype(np.float64)
    return np.array(finish_loss(S, D, Ncnt, preds.shape[0] * HW), dtype=np.float32)



# revision 2
# speedup vs baseline: 2.0806x; 2.0806x over previous
"""DiceLoss Trainium2 kernel — rotated/binned softmax design.

Math: preds [B,C,H,W] logits, targets [B,H,W] ints; P = softmax over C.
The loss needs only the 32-vectors S_c = sum_n P_nc and D_c = sum_{t_n=c}
P_nc (plus counts). Decomposition (per core = one batch):

 - pixels are binned into (group, class) cells: 4 groups x 32 classes,
   each cell <= 2048 pixels ("main" runs); per-class overflow goes to
   small fixed-width "tail" runs. Cells are class-pure.
 - HOST ROTATION: for a pixel with target c placed in a cell of class c,
   partition row (32g+i) holds logit of class (c+i)%32. Row order within
   a pixel's column is irrelevant for its softmax denominator Z, so Z is
   still computable with a block-diagonal ones matmul; and the OWN-class
   probability always lands in slot i=0.
 - device per tile (one run, [128,K]): E=exp(x) [ACT], Z=blockdiag@E
   [PE->PSUM], then ONE fused custom DVE op:
       sigma[p, run] = sum_k E[p,k] * recip_1nr(Z[p,k])
   (reciprocal = BITWISE_NOT-seeded 1-step Newton, ~0.17% worst rel err).
 - host: slot i of run (class c) contributes to S[(c+i)%32]; slot 0 is
   D[c]. Pad columns are exp-known and subtracted exactly.
"""

import re
import numpy as np
import ml_dtypes
from operator import add

import concourse.bass as bass
import concourse.bacc as bacc
import concourse.mybir as mybir
from concourse.tile import TileContext
from concourse.bass_utils import run_bass_kernel_spmd

# ---- fused custom DVE op: accum[p] += in0[p,k] * recip_approx(in1[p,k]) ----


def _make_div_reduce_op():
    import concourse.dve_ops as dve_ops
    from concourse.dve_ops import DveOp
    from concourse.dve_spec import Spec, Src0, Src1, C0, C1, Zero, Bin
    from concourse.dve_uop import AluOp

    name = "DICE_DIV_REDUCE"
    if name in dve_ops._SUB_OPCODE_FOR_NAME:
        for op in dve_ops.OPS:
            if op.name == name:
                return op

    _nx = Bin(AluOp.BITWISE_NOT, Src1, Src1)
    _w0 = _nx * C0
    _w1 = _w0 * (C1 - Src1 * _w0)

    def _ref(in0, in1, c0, c1, imm2):
        nx = (~in1.view(np.int32)).view(np.float32)
        y0 = nx * np.float32(c0)
        y1 = y0 * (np.float32(c1) - in1 * y0)
        b = (in0.astype(np.float32) * y1).astype(np.float32)
        return b, b.reshape(b.shape[0], -1).sum(axis=-1, keepdims=True)

    spec = Spec(body=Src0 * _w1, accum=add, accum_init=Zero, reference=_ref)
    row = dve_ops._CUSTOM_DVE_ROW_BASE + len(dve_ops.OPS)
    assert row < 0x20
    op = DveOp(name, spec, subdim=False, uops_sha={})
    dve_ops.OPS.append(op)
    dve_ops.CUSTOM_DVE_SPECS[name] = spec
    dve_ops._SUB_OPCODE_FOR_NAME[name] = row
    for ver in ("v3", "v4"):
        try:
            op.compile(ver)
        except ValueError as e:
            m = re.search(r'uops_sha\["%s"\]="([0-9a-f]+)"' % ver, str(e))
            if not m:
                raise
            op.uops_sha[ver] = m.group(1)
            dve_ops._COMPILE_CACHE.pop((name, ver), None)
        op.compile(ver)
    return op


DICE_DIV_REDUCE = _make_div_reduce_op()

# Chebyshev seed constants (shared with RECIPROCAL_APPROX_FAST).
RC0 = -0.23549792
RC1 = 2.0017324


def _recip1nr_host(z):
    z = np.asarray(z, dtype=np.float32)
    nx = (~z.view(np.int32)).view(np.float32)
    y0 = nx * np.float32(RC0)
    return y0 * (np.float32(RC1) - z * y0)


# ---- problem constants ------------------------------------------------------
B, C, H, W = 8, 32, 512, 512
HW = H * W
G = 4
CAP = 2048               # main cell capacity == main run width
EPS = 1e-8
SMOOTH = 1e-5
NCORES = 8
BIGNEG = -30.0

F32 = mybir.dt.float32
BF16 = mybir.dt.bfloat16
BF = ml_dtypes.bfloat16


# ---- device program ---------------------------------------------------------

def build_nc(nt2, slot2):
    """One-core SPMD program. 32 main runs of CAP cols + nt2 tail runs of
    slot2 cols. sig_out[:, r] = sum over run r of E * recip(Z)."""
    tot = 32 * CAP + nt2 * slot2
    nruns = 32 + nt2
    nc = bacc.Bacc("TRN2", target_bir_lowering=False)
    x = nc.declare_dram_parameter("x", [128, tot], BF16, isOutput=False)
    w1 = nc.declare_dram_parameter("w1", [128, 128], BF16, isOutput=False)
    sig_out = nc.declare_dram_parameter("sig_out", [128, nruns], F32, isOutput=True)

    MMF = 512
    with TileContext(nc) as tc:
        with (
            tc.tile_pool(name="const", bufs=1) as constp,
            tc.tile_pool(name="xin", bufs=6) as xp,
            tc.tile_pool(name="ework", bufs=6) as ep,
            tc.tile_pool(name="junk", bufs=4) as jp,
            tc.tile_pool(name="acc", bufs=1) as accp,
            tc.tile_pool(name="ps1", bufs=2, space="PSUM") as ps1,
        ):
            w1_t = constp.tile([128, 128], BF16)
            nc.sync.dma_start(out=w1_t[:], in_=w1[:])
            sig = accp.tile([128, nruns], F32)

            def run_tile(col0, k, r):
                xt = xp.tile([128, k], BF16, tag="x")
                nc.sync.dma_start(out=xt[:], in_=x[:, col0:col0 + k])
                et = ep.tile([128, k], BF16, tag="e")
                nc.scalar.activation(et[:], xt[:], mybir.ActivationFunctionType.Exp)
                z_ps = ps1.tile([128, k], F32, tag="z")
                for m0 in range(0, k, MMF):
                    m1 = min(m0 + MMF, k)
                    nc.tensor.matmul(z_ps[:, m0:m1], w1_t[:], et[:, m0:m1],
                                     start=True, stop=True)
                j = jp.tile([128, 1], BF16, tag="j")
                nc.vector._custom_dve(
                    DICE_DIV_REDUCE, out=j[:].broadcast_to((128, k)),
                    in0=et[:], in1=z_ps[:],
                    s0=RC0, s1=RC1, imm2=0.0,
                    accum_out=sig[:, r:r + 1])

            for t in range(32):
                run_tile(t * CAP, CAP, t)
            for t in range(nt2):
                run_tile(32 * CAP + t * slot2, slot2, 32 + t)

            nc.sync.dma_start(out=sig_out[:], in_=sig[:])
    nc.finalize()
    return nc


_NC_CACHE = {}


def _get_nc(nt2=None, slot2=None):
    if nt2 is None:
        # test.py calls _get_nc() with no args after kernel() has run
        key = next(iter(_NC_CACHE))
        return _NC_CACHE[key]
    key = (nt2, slot2)
    if key not in _NC_CACHE:
        _NC_CACHE[key] = build_nc(nt2, slot2)
    return _NC_CACHE[key]


def host_w1():
    w1 = np.zeros((128, 128), dtype=BF)
    for g in range(G):
        w1[g * 32:(g + 1) * 32, g * 32:(g + 1) * 32] = BF(1.0)
    return w1


# ---- host prep --------------------------------------------------------------

def plan_core(t_flat):
    """Returns (main_cells, tail_cells): main_cells[g][c] = pixel idx array
    (<= CAP); tail_cells = list of (class, idx)."""
    order = np.argsort(t_flat, kind="stable")
    t_sorted = t_flat[order]
    starts = np.searchsorted(t_sorted, np.arange(C))
    ends = np.searchsorted(t_sorted, np.arange(C), side="right")
    main_cells = [[None] * C for _ in range(G)]
    tails = []
    for c in range(C):
        idx = order[starts[c]:ends[c]]
        n = idx.shape[0]
        q = min(n, G * CAP)
        base, rem = divmod(q, G)
        pos = 0
        for g in range(G):
            take = base + (1 if g < rem else 0)
            main_cells[g][c] = idx[pos:pos + take]
            pos += take
        if n > q:
            tails.append((c, idx[q:]))
    return main_cells, tails


def fill_region(xp_out, X, cells_by_group, sizes, col_base):
    """cells_by_group[g][r] = (class, idx); sizes[r] = run width.
    Fills xp_out (f32, init'd) and returns (cmap [G,nr], padcnt [G,nr])."""
    nr = len(sizes)
    cmap = np.zeros((G, nr), dtype=np.int64)
    padcnt = np.zeros((G, nr), dtype=np.int64)
    off = col_base
    for r in range(nr):
        L = sizes[r]
        for g in range(G):
            c, idx = cells_by_group[g][r]
            cmap[g, r] = c
            n = idx.shape[0]
            padcnt[g, r] = L - n
            if n:
                rot_rows = (c + np.arange(C)) % C
                xp_out[32 * g:32 * g + 32, off:off + n] = \
                    X[rot_rows[:, None], idx[None, :]]
        off += L
    return cmap, padcnt


def finish_loss(S, D, Ncnt, npix):
    TP = EPS * S + (1.0 - EPS) * D
    FP = S - TP
    FN = (EPS * npix + (1.0 - EPS) * Ncnt) - TP
    alpha = np.clip(FP / (FP + FN + SMOOTH), 0.2, 0.8)
    beta = 1.0 - alpha
    den = TP + alpha * FP + beta * FN
    dice = TP / (den + SMOOTH)
    return np.float32(np.sum(1.0 - dice) / C)


def kernel(preds, targets):
    preds = np.asarray(preds, dtype=np.float32)
    targets = np.asarray(targets)

    plans = []
    max_tail_cells = 0
    max_tail_len = 1
    for b in range(NCORES):
        t_flat = targets[b].reshape(-1).astype(np.int64)
        main_cells, tails = plan_core(t_flat)
        plans.append((main_cells, tails))
        max_tail_cells = max(max_tail_cells, len(tails))
        for _, idx in tails:
            max_tail_len = max(max_tail_len, idx.shape[0])

    nt2 = (max_tail_cells + G - 1) // G
    slot2 = ((max_tail_len + 63) // 64) * 64
    tot = 32 * CAP + nt2 * slot2

    nc = _get_nc(nt2, slot2)
    w1 = host_w1()

    in_maps = []
    metas = []
    for b in range(NCORES):
        main_cells, tails = plans[b]
        X = preds[b].reshape(C, HW)
        xp = np.full((128, tot), np.float32(BIGNEG), dtype=np.float32)
        xp[0::32, :] = 0.0  # slot-0 rows default 0 (pad columns)
        mains = [[(c, main_cells[g][c]) for c in range(C)] for g in range(G)]
        cmap_m, pad_m = fill_region(xp, X, mains, [CAP] * C, 0)
        # tails round-robin into (group, run) cells
        tcell = [[(0, np.zeros(0, dtype=np.int64)) for _ in range(nt2)]
                 for _ in range(G)]
        for j, (c, idx) in enumerate(tails):
            tcell[j % G][j // G] = (c, idx)
        cmap_t, pad_t = fill_region(xp, X, tcell, [slot2] * nt2, 32 * CAP)
        in_maps.append({"x": xp.astype(BF), "w1": w1})
        metas.append((cmap_m, pad_m, cmap_t, pad_t))

    res = run_bass_kernel_spmd(nc, in_maps, list(range(NCORES))).results

    # pad column contribution as the device computes it: slot0 ~ 1*recip(1),
    # other slots ~ exp(BIGNEG) (negligible but subtracted anyway).
    p_pad = np.full(C, np.exp(np.float64(BIGNEG)))
    p_pad[0] = np.float64(_recip1nr_host(1.0))

    S = np.zeros(C, dtype=np.float64)
    D = np.zeros(C, dtype=np.float64)
    ii = np.arange(C)
    for b in range(NCORES):
        sig = np.asarray(res[b]["sig_out"], dtype=np.float64)  # [128, nruns]
        cmap_m, pad_m, cmap_t, pad_t = metas[b]
        for region, cmap, pad in ((sig[:, :C], cmap_m, pad_m),
                                  (sig[:, C:], cmap_t, pad_t)):
            for g in range(G):
                blk = region[32 * g:32 * g + 32, :]  # [slot i, run r]
                corr = blk - np.outer(p_pad, pad[g])
                for r in range(cmap.shape[1]):
                    c = cmap[g, r]
                    np.add.at(S, (c + ii) % C, corr[:, r])
                    D[c] += corr[0, r]

    Ncnt = np.bincount(targets.reshape(-1).astype(np.int64),
                       minlength=C).astype(np.float64)
    return np.array(finish_loss(S, D, Ncnt, preds.shape[0] * HW),
                    dtype=np.float32)


# revision 13
# speedup vs baseline: 2.1881x; 1.0517x over previous
"""DiceLoss Trainium2 kernel — rotated/binned softmax design.

Math: preds [B,C,H,W] logits, targets [B,H,W] ints; P = softmax over C.
The loss needs only the 32-vectors S_c = sum_n P_nc and D_c = sum_{t_n=c}
P_nc (plus counts). Decomposition (per core = one batch):

 - pixels are binned into (group, class) cells: 4 groups x 32 classes,
   each cell <= 2048 pixels ("main" runs); per-class overflow goes to
   small fixed-width "tail" runs. Cells are class-pure.
 - HOST ROTATION: for a pixel with target c placed in a cell of class c,
   partition row (32g+i) holds logit of class (c+i)%32. Row order within
   a pixel's column is irrelevant for its softmax denominator Z, so Z is
   still computable with a block-diagonal ones matmul; and the OWN-class
   probability always lands in slot i=0.
 - device per tile (one run, [128,K]): E=exp(x) [ACT], Z=blockdiag@E
   [PE->PSUM], then ONE fused custom DVE op:
       sigma[p, run] = sum_k E[p,k] * recip_1nr(Z[p,k])
   (reciprocal = BITWISE_NOT-seeded 1-step Newton, ~0.17% worst rel err).
 - host: slot i of run (class c) contributes to S[(c+i)%32]; slot 0 is
   D[c]. Pad columns are exp-known and subtracted exactly.
"""

import re
import numpy as np
import ml_dtypes
from operator import add

import concourse.bass as bass
import concourse.bacc as bacc
import concourse.mybir as mybir
from concourse.tile import TileContext
from concourse.bass_utils import run_bass_kernel_spmd

# ---- fused custom DVE op: accum[p] += in0[p,k] * recip_approx(in1[p,k]) ----


def _make_div_reduce_op():
    import concourse.dve_ops as dve_ops
    from concourse.dve_ops import DveOp
    from concourse.dve_spec import Spec, Src0, Src1, C0, C1, Zero, Bin
    from concourse.dve_uop import AluOp

    name = "DICE_DIV_REDUCE"
    if name in dve_ops._SUB_OPCODE_FOR_NAME:
        for op in dve_ops.OPS:
            if op.name == name:
                return op

    _nx = Bin(AluOp.BITWISE_NOT, Src1, Src1)
    _w0 = _nx * C0
    _w1 = _w0 * (C1 - Src1 * _w0)

    def _ref(in0, in1, c0, c1, imm2):
        nx = (~in1.view(np.int32)).view(np.float32)
        y0 = nx * np.float32(c0)
        y1 = y0 * (np.float32(c1) - in1 * y0)
        b = (in0.astype(np.float32) * y1).astype(np.float32)
        return b, b.reshape(b.shape[0], -1).sum(axis=-1, keepdims=True)

    spec = Spec(body=Src0 * _w1, accum=add, accum_init=Zero, reference=_ref)
    row = dve_ops._CUSTOM_DVE_ROW_BASE + len(dve_ops.OPS)
    assert row < 0x20
    op = DveOp(name, spec, subdim=False, uops_sha={})
    dve_ops.OPS.append(op)
    dve_ops.CUSTOM_DVE_SPECS[name] = spec
    dve_ops._SUB_OPCODE_FOR_NAME[name] = row
    for ver in ("v3", "v4"):
        try:
            op.compile(ver)
        except ValueError as e:
            m = re.search(r'uops_sha\["%s"\]="([0-9a-f]+)"' % ver, str(e))
            if not m:
                raise
            op.uops_sha[ver] = m.group(1)
            dve_ops._COMPILE_CACHE.pop((name, ver), None)
        op.compile(ver)
    return op


DICE_DIV_REDUCE = _make_div_reduce_op()

# Chebyshev seed constants (shared with RECIPROCAL_APPROX_FAST).
RC0 = -0.23549792
RC1 = 2.0017324


def _recip1nr_host(z):
    z = np.asarray(z, dtype=np.float32)
    nx = (~z.view(np.int32)).view(np.float32)
    y0 = nx * np.float32(RC0)
    return y0 * (np.float32(RC1) - z * y0)


# ---- problem constants ------------------------------------------------------
B, C, H, W = 8, 32, 512, 512
HW = H * W
G = 4
CAP = 2048               # main cell capacity == main run width
EPS = 1e-8
SMOOTH = 1e-5
NCORES = 8
BIGNEG = -30.0

F32 = mybir.dt.float32
BF16 = mybir.dt.bfloat16
BF = ml_dtypes.bfloat16


# ---- device program ---------------------------------------------------------

# Warmup schedule: run 0 is split into small sub-runs so the DMA->ACT->PE->
# DVE pipeline fills fast (the first DVE op can start after a ~256-col
# chain instead of a 2048-col one). Extra accum columns are summed on host.
WARMUP = {0: [256, 768, 1024], 1: [768, 1280], 2: [1024, 1024], 3: [1024, 1024]}
# sig layout: warmup sub-run cols first (in run order), then full runs.
NWARM = sum(len(v) for v in WARMUP.values())
NSIG = NWARM + 32 - len(WARMUP)


def build_nc():
    """One-core SPMD program: 32 class-pure runs of CAP cols.
    sig_out[:, r] = sum over run r of E * recip_1nr(Z)."""
    tot = 32 * CAP
    nc = bacc.Bacc("TRN2", target_bir_lowering=False)
    x = nc.declare_dram_parameter("x", [128, tot], BF16, isOutput=False)
    w1 = nc.declare_dram_parameter("w1", [128, 128], BF16, isOutput=False)
    sig_out = nc.declare_dram_parameter("sig_out", [128, NSIG], F32, isOutput=True)

    MMF = 512
    with TileContext(nc) as tc:
        with (
            tc.tile_pool(name="const", bufs=1) as constp,
            tc.tile_pool(name="xin", bufs=6) as xp,
            tc.tile_pool(name="ework", bufs=6) as ep,
            tc.tile_pool(name="junk", bufs=4) as jp,
            tc.tile_pool(name="acc", bufs=1) as accp,
            tc.tile_pool(name="ps1", bufs=2, space="PSUM") as ps1,
        ):
            w1_t = constp.tile([128, 128], BF16)
            nc.sync.dma_start(out=w1_t[:], in_=w1[:])
            sig = accp.tile([128, NSIG], F32)

            def run_tile(col0, k, r):
                xt = xp.tile([128, k], BF16, tag="x")
                nc.sync.dma_start(out=xt[:], in_=x[:, col0:col0 + k])
                et = ep.tile([128, k], BF16, tag="e")
                nc.scalar.activation(et[:], xt[:], mybir.ActivationFunctionType.Exp)
                z_big = ps1.tile([128, CAP], F32, tag="z")
                z_ps = z_big[:, :k]
                for m0 in range(0, k, MMF):
                    m1 = min(m0 + MMF, k)
                    nc.tensor.matmul(z_ps[:, m0:m1], w1_t[:], et[:, m0:m1],
                                     start=True, stop=True)
                j = jp.tile([128, 1], BF16, tag="j")
                nc.vector._custom_dve(
                    DICE_DIV_REDUCE, out=j[:].broadcast_to((128, k)),
                    in0=et[:], in1=z_ps[:],
                    s0=RC0, s1=RC1, imm2=0.0,
                    accum_out=sig[:, r:r + 1])

            r = 0
            for t in range(32):
                if t in WARMUP:
                    col = t * CAP
                    for k in WARMUP[t]:
                        run_tile(col, k, r)
                        col += k
                        r += 1
                    assert col == (t + 1) * CAP
                else:
                    run_tile(t * CAP, CAP, r)
                    r += 1

            nc.sync.dma_start(out=sig_out[:], in_=sig[:])
    nc.finalize()
    return nc


_NC_CACHE = {}


def _get_nc():
    if "nc" not in _NC_CACHE:
        _NC_CACHE["nc"] = build_nc()
    return _NC_CACHE["nc"]


def host_w1():
    w1 = np.zeros((128, 128), dtype=BF)
    for g in range(G):
        w1[g * 32:(g + 1) * 32, g * 32:(g + 1) * 32] = BF(1.0)
    return w1


# ---- host prep --------------------------------------------------------------

def plan_core(t_flat):
    """Returns (main_cells, tail_cells): main_cells[g][c] = pixel idx array
    (<= CAP); tail_cells = list of (class, idx)."""
    order = np.argsort(t_flat, kind="stable")
    t_sorted = t_flat[order]
    starts = np.searchsorted(t_sorted, np.arange(C))
    ends = np.searchsorted(t_sorted, np.arange(C), side="right")
    main_cells = [[None] * C for _ in range(G)]
    tails = []
    for c in range(C):
        idx = order[starts[c]:ends[c]]
        n = idx.shape[0]
        q = min(n, G * CAP)
        base, rem = divmod(q, G)
        pos = 0
        for g in range(G):
            take = base + (1 if g < rem else 0)
            main_cells[g][c] = idx[pos:pos + take]
            pos += take
        if n > q:
            tails.append((c, idx[q:]))
    return main_cells, tails


def fill_region(xp_out, X, cells_by_group, sizes, col_base):
    """cells_by_group[g][r] = (class, idx); sizes[r] = run width.
    Fills xp_out (f32, init'd) and returns (cmap [G,nr], padcnt [G,nr])."""
    nr = len(sizes)
    cmap = np.zeros((G, nr), dtype=np.int64)
    padcnt = np.zeros((G, nr), dtype=np.int64)
    off = col_base
    for r in range(nr):
        L = sizes[r]
        for g in range(G):
            c, idx = cells_by_group[g][r]
            cmap[g, r] = c
            n = idx.shape[0]
            padcnt[g, r] = L - n
            if n:
                rot_rows = (c + np.arange(C)) % C
                xp_out[32 * g:32 * g + 32, off:off + n] = \
                    X[rot_rows[:, None], idx[None, :]]
        off += L
    return cmap, padcnt


def finish_loss(S, D, Ncnt, npix):
    TP = EPS * S + (1.0 - EPS) * D
    FP = S - TP
    FN = (EPS * npix + (1.0 - EPS) * Ncnt) - TP
    alpha = np.clip(FP / (FP + FN + SMOOTH), 0.2, 0.8)
    beta = 1.0 - alpha
    den = TP + alpha * FP + beta * FN
    dice = TP / (den + SMOOTH)
    return np.float32(np.sum(1.0 - dice) / C)


def host_tail_SD(X, tails):
    """Exact float64 softmax S/D contributions for overflow pixels (the
    <0.5% of pixels beyond the 4x2048 per-class device cells)."""
    S = np.zeros(C, dtype=np.float64)
    D = np.zeros(C, dtype=np.float64)
    for c, idx in tails:
        lg = X[:, idx].astype(np.float64)          # [C, n]
        m = lg.max(axis=0, keepdims=True)
        e = np.exp(lg - m)
        P = e / e.sum(axis=0, keepdims=True)
        S += P.sum(axis=1)
        D[c] += P[c].sum()
    return S, D


def kernel(preds, targets):
    preds = np.asarray(preds, dtype=np.float32)
    targets = np.asarray(targets)

    nc = _get_nc()
    w1 = host_w1()
    tot = 32 * CAP

    S = np.zeros(C, dtype=np.float64)
    D = np.zeros(C, dtype=np.float64)

    in_maps = []
    metas = []
    for b in range(NCORES):
        t_flat = targets[b].reshape(-1).astype(np.int64)
        main_cells, tails = plan_core(t_flat)
        X = preds[b].reshape(C, HW)
        xp = np.full((128, tot), np.float32(BIGNEG), dtype=np.float32)
        xp[0::32, :] = 0.0  # slot-0 rows default 0 (pad columns)
        mains = [[(c, main_cells[g][c]) for c in range(C)] for g in range(G)]
        cmap_m, pad_m = fill_region(xp, X, mains, [CAP] * C, 0)
        in_maps.append({"x": xp.astype(BF), "w1": w1})
        metas.append((cmap_m, pad_m))
        if tails:
            St, Dt = host_tail_SD(X, tails)
            S += St
            D += Dt

    res = run_bass_kernel_spmd(nc, in_maps, list(range(NCORES))).results

    # pad column contribution as the device computes it: slot0 ~ 1*recip(1),
    # other slots ~ exp(BIGNEG) (negligible but subtracted anyway).
    p_pad = np.full(C, np.exp(np.float64(BIGNEG)))
    p_pad[0] = np.float64(_recip1nr_host(1.0))

    ii = np.arange(C)
    # map sig columns back to runs (warmup sub-runs fold into their run)
    colmap = []
    for t in range(32):
        colmap.extend([t] * len(WARMUP.get(t, [0])))
    colmap = np.asarray(colmap)
    assert colmap.shape[0] == NSIG
    for b in range(NCORES):
        sig = np.asarray(res[b]["sig_out"], dtype=np.float64)  # [128, NSIG]
        sigf = np.zeros((128, C), dtype=np.float64)
        np.add.at(sigf.T, colmap, sig.T)
        cmap_m, pad_m = metas[b]
        for g in range(G):
            blk = sigf[32 * g:32 * g + 32, :]  # [slot i, run r]
            corr = blk - np.outer(p_pad, pad_m[g])
            for r in range(C):
                c = cmap_m[g, r]
                np.add.at(S, (c + ii) % C, corr[:, r])
                D[c] += corr[0, r]

    Ncnt = np.bincount(targets.reshape(-1).astype(np.int64),
                       minlength=C).astype(np.float64)
    return np.array(finish_loss(S, D, Ncnt, preds.shape[0] * HW),
                    dtype=np.float32)


# revision 15
# speedup vs baseline: 2.2004x; 1.0056x over previous
"""DiceLoss Trainium2 kernel — rotated/binned softmax design.

Math: preds [B,C,H,W] logits, targets [B,H,W] ints; P = softmax over C.
The loss needs only the 32-vectors S_c = sum_n P_nc and D_c = sum_{t_n=c}
P_nc (plus counts). Decomposition (per core = one batch):

 - pixels are binned into class-pure (group, run) cells: 4 groups x 32
   runs of 2048 columns (exactly one PSUM double-buffer rotation). The
   <0.5% per-class overflow beyond 4x2048 is finished on the host in
   float64 (exact), keeping the device program a single fixed shape.
 - HOST ROTATION: for a pixel with target c placed in a cell of class c,
   partition row (32g+i) holds logit of class (c+i)%32. Row order within
   a pixel's column is irrelevant for its softmax denominator Z, so Z is
   still computable with a block-diagonal ones matmul; and the OWN-class
   probability always lands in slot i=0, so ONE reduce per run yields
   both S (all slots) and D (slot 0) — no second masked reduce needed.
 - device per run [128,K]: E=exp(x) [ACT], Z=blockdiag@E [PE->PSUM],
   then ONE fused custom DVE op:
       sigma[p, run] = sum_k E[p,k] * recip_1nr(Z[p,k])
   (reciprocal = BITWISE_NOT-seeded 1-step Newton, ~0.17% worst rel err).
 - host: slot i of run (class c) contributes to S[(c+i)%32]; slot 0 is
   D[c]. Pad columns are exp-known and subtracted exactly.
"""

import re
import numpy as np
import ml_dtypes
from operator import add

import concourse.bacc as bacc
import concourse.mybir as mybir
from concourse.tile import TileContext
from concourse.bass_utils import run_bass_kernel_spmd

# ---- fused custom DVE op: accum[p] += in0[p,k] * recip_approx(in1[p,k]) ----


def _make_div_reduce_op():
    import concourse.dve_ops as dve_ops
    from concourse.dve_ops import DveOp
    from concourse.dve_spec import Spec, Src0, Src1, C0, C1, Zero, Bin
    from concourse.dve_uop import AluOp

    name = "DICE_DIV_REDUCE"
    if name in dve_ops._SUB_OPCODE_FOR_NAME:
        for op in dve_ops.OPS:
            if op.name == name:
                return op

    _nx = Bin(AluOp.BITWISE_NOT, Src1, Src1)
    _w0 = _nx * C0
    _w1 = _w0 * (C1 - Src1 * _w0)

    def _ref(in0, in1, c0, c1, imm2):
        nx = (~in1.view(np.int32)).view(np.float32)
        y0 = nx * np.float32(c0)
        y1 = y0 * (np.float32(c1) - in1 * y0)
        b = (in0.astype(np.float32) * y1).astype(np.float32)
        return b, b.reshape(b.shape[0], -1).sum(axis=-1, keepdims=True)

    spec = Spec(body=Src0 * _w1, accum=add, accum_init=Zero, reference=_ref)
    row = dve_ops._CUSTOM_DVE_ROW_BASE + len(dve_ops.OPS)
    assert row < 0x20
    op = DveOp(name, spec, subdim=False, uops_sha={})
    dve_ops.OPS.append(op)
    dve_ops.CUSTOM_DVE_SPECS[name] = spec
    dve_ops._SUB_OPCODE_FOR_NAME[name] = row
    for ver in ("v3", "v4"):
        try:
            op.compile(ver)
        except ValueError as e:
            m = re.search(r'uops_sha\["%s"\]="([0-9a-f]+)"' % ver, str(e))
            if not m:
                raise
            op.uops_sha[ver] = m.group(1)
            dve_ops._COMPILE_CACHE.pop((name, ver), None)
        op.compile(ver)
    return op


DICE_DIV_REDUCE = _make_div_reduce_op()

# Chebyshev seed constants (shared with RECIPROCAL_APPROX_FAST).
RC0 = -0.23549792
RC1 = 2.0017324


def _recip1nr_host(z):
    z = np.asarray(z, dtype=np.float32)
    nx = (~z.view(np.int32)).view(np.float32)
    y0 = nx * np.float32(RC0)
    return y0 * (np.float32(RC1) - z * y0)


# ---- problem constants ------------------------------------------------------
B, C, H, W = 8, 32, 512, 512
HW = H * W
G = 4
CAP = 2048               # main cell capacity == main run width
EPS = 1e-8
SMOOTH = 1e-5
NCORES = 8
BIGNEG = -30.0

F32 = mybir.dt.float32
BF16 = mybir.dt.bfloat16
BF = ml_dtypes.bfloat16


# ---- device program ---------------------------------------------------------

# Warmup schedule: the first few runs are split into smaller sub-runs so
# the DMA->ACT->PE->DVE pipeline fills fast (the first DVE op starts after
# a ~256-col chain instead of a 2048-col one). Extra accum columns are
# folded back into their run on the host.
WARMUP = {0: [256, 768, 1024], 1: [768, 1280], 2: [1024, 1024], 3: [1024, 1024]}
# sig layout: warmup sub-run cols first (in run order), then full runs.
NWARM = sum(len(v) for v in WARMUP.values())
NSIG = NWARM + 32 - len(WARMUP)


def build_nc():
    """One-core SPMD program: 32 class-pure runs of CAP cols.
    sig_out[:, r] = sum over run r of E * recip_1nr(Z)."""
    tot = 32 * CAP
    nc = bacc.Bacc("TRN2", target_bir_lowering=False)
    x = nc.declare_dram_parameter("x", [128, tot], BF16, isOutput=False)
    w1 = nc.declare_dram_parameter("w1", [128, 128], BF16, isOutput=False)
    sig_out = nc.declare_dram_parameter("sig_out", [128, NSIG], F32, isOutput=True)

    MMF = 512
    with TileContext(nc) as tc:
        with (
            tc.tile_pool(name="const", bufs=1) as constp,
            tc.tile_pool(name="xin", bufs=6) as xp,
            tc.tile_pool(name="ework", bufs=6) as ep,
            tc.tile_pool(name="junk", bufs=4) as jp,
            tc.tile_pool(name="acc", bufs=1) as accp,
            tc.tile_pool(name="ps1", bufs=2, space="PSUM") as ps1,
        ):
            w1_t = constp.tile([128, 128], BF16)
            nc.sync.dma_start(out=w1_t[:], in_=w1[:])
            sig = accp.tile([128, NSIG], F32)

            def run_tile(col0, k, r):
                xt = xp.tile([128, k], BF16, tag="x")
                nc.sync.dma_start(out=xt[:], in_=x[:, col0:col0 + k])
                et = ep.tile([128, k], BF16, tag="e")
                nc.scalar.activation(et[:], xt[:], mybir.ActivationFunctionType.Exp)
                z_big = ps1.tile([128, CAP], F32, tag="z")
                z_ps = z_big[:, :k]
                for m0 in range(0, k, MMF):
                    m1 = min(m0 + MMF, k)
                    nc.tensor.matmul(z_ps[:, m0:m1], w1_t[:], et[:, m0:m1],
                                     start=True, stop=True)
                j = jp.tile([128, 1], BF16, tag="j")
                nc.vector._custom_dve(
                    DICE_DIV_REDUCE, out=j[:].broadcast_to((128, k)),
                    in0=et[:], in1=z_ps[:],
                    s0=RC0, s1=RC1, imm2=0.0,
                    accum_out=sig[:, r:r + 1])

            r = 0
            for t in range(32):
                if t in WARMUP:
                    col = t * CAP
                    for k in WARMUP[t]:
                        run_tile(col, k, r)
                        col += k
                        r += 1
                    assert col == (t + 1) * CAP
                else:
                    run_tile(t * CAP, CAP, r)
                    r += 1

            nc.sync.dma_start(out=sig_out[:], in_=sig[:])
    nc.finalize()
    return nc


_NC_CACHE = {}


def _get_nc():
    if "nc" not in _NC_CACHE:
        _NC_CACHE["nc"] = build_nc()
    return _NC_CACHE["nc"]


def host_w1():
    w1 = np.zeros((128, 128), dtype=BF)
    for g in range(G):
        w1[g * 32:(g + 1) * 32, g * 32:(g + 1) * 32] = BF(1.0)
    return w1


# ---- host prep --------------------------------------------------------------

def plan_core(t_flat):
    """Returns (main_cells, tail_cells): main_cells[g][c] = pixel idx array
    (<= CAP); tail_cells = list of (class, idx)."""
    order = np.argsort(t_flat, kind="stable")
    t_sorted = t_flat[order]
    starts = np.searchsorted(t_sorted, np.arange(C))
    ends = np.searchsorted(t_sorted, np.arange(C), side="right")
    main_cells = [[None] * C for _ in range(G)]
    tails = []
    for c in range(C):
        idx = order[starts[c]:ends[c]]
        n = idx.shape[0]
        q = min(n, G * CAP)
        base, rem = divmod(q, G)
        pos = 0
        for g in range(G):
            take = base + (1 if g < rem else 0)
            main_cells[g][c] = idx[pos:pos + take]
            pos += take
        if n > q:
            tails.append((c, idx[q:]))
    return main_cells, tails


def fill_region(xp_out, X, cells_by_group, sizes, col_base):
    """cells_by_group[g][r] = (class, idx); sizes[r] = run width.
    Fills xp_out (f32, init'd) and returns (cmap [G,nr], padcnt [G,nr])."""
    nr = len(sizes)
    cmap = np.zeros((G, nr), dtype=np.int64)
    padcnt = np.zeros((G, nr), dtype=np.int64)
    off = col_base
    for r in range(nr):
        L = sizes[r]
        for g in range(G):
            c, idx = cells_by_group[g][r]
            cmap[g, r] = c
            n = idx.shape[0]
            padcnt[g, r] = L - n
            if n:
                rot_rows = (c + np.arange(C)) % C
                xp_out[32 * g:32 * g + 32, off:off + n] = \
                    X[rot_rows[:, None], idx[None, :]]
        off += L
    return cmap, padcnt


def finish_loss(S, D, Ncnt, npix):
    TP = EPS * S + (1.0 - EPS) * D
    FP = S - TP
    FN = (EPS * npix + (1.0 - EPS) * Ncnt) - TP
    alpha = np.clip(FP / (FP + FN + SMOOTH), 0.2, 0.8)
    beta = 1.0 - alpha
    den = TP + alpha * FP + beta * FN
    dice = TP / (den + SMOOTH)
    return np.float32(np.sum(1.0 - dice) / C)


def host_tail_SD(X, tails):
    """Exact float64 softmax S/D contributions for overflow pixels (the
    <0.5% of pixels beyond the 4x2048 per-class device cells)."""
    S = np.zeros(C, dtype=np.float64)
    D = np.zeros(C, dtype=np.float64)
    for c, idx in tails:
        lg = X[:, idx].astype(np.float64)          # [C, n]
        m = lg.max(axis=0, keepdims=True)
        e = np.exp(lg - m)
        P = e / e.sum(axis=0, keepdims=True)
        S += P.sum(axis=1)
        D[c] += P[c].sum()
    return S, D


def kernel(preds, targets):
    preds = np.asarray(preds, dtype=np.float32)
    targets = np.asarray(targets)

    nc = _get_nc()
    w1 = host_w1()
    tot = 32 * CAP

    S = np.zeros(C, dtype=np.float64)
    D = np.zeros(C, dtype=np.float64)

    in_maps = []
    metas = []
    for b in range(NCORES):
        t_flat = targets[b].reshape(-1).astype(np.int64)
        main_cells, tails = plan_core(t_flat)
        X = preds[b].reshape(C, HW)
        xp = np.full((128, tot), np.float32(BIGNEG), dtype=np.float32)
        xp[0::32, :] = 0.0  # slot-0 rows default 0 (pad columns)
        mains = [[(c, main_cells[g][c]) for c in range(C)] for g in range(G)]
        cmap_m, pad_m = fill_region(xp, X, mains, [CAP] * C, 0)
        in_maps.append({"x": xp.astype(BF), "w1": w1})
        metas.append((cmap_m, pad_m))
        if tails:
            St, Dt = host_tail_SD(X, tails)
            S += St
            D += Dt

    res = run_bass_kernel_spmd(nc, in_maps, list(range(NCORES))).results

    # pad column contribution as the device computes it: slot0 ~ 1*recip(1),
    # other slots ~ exp(BIGNEG) (negligible but subtracted anyway).
    p_pad = np.full(C, np.exp(np.float64(BIGNEG)))
    p_pad[0] = np.float64(_recip1nr_host(1.0))

    ii = np.arange(C)
    # map sig columns back to runs (warmup sub-runs fold into their run)
    colmap = []
    for t in range(32):
        colmap.extend([t] * len(WARMUP.get(t, [0])))
    colmap = np.asarray(colmap)
    assert colmap.shape[0] == NSIG
    for b in range(NCORES):
        sig = np.asarray(res[b]["sig_out"], dtype=np.float64)  # [128, NSIG]
        sigf = np.zeros((128, C), dtype=np.float64)
        np.add.at(sigf.T, colmap, sig.T)
        cmap_m, pad_m = metas[b]
        for g in range(G):
            blk = sigf[32 * g:32 * g + 32, :]  # [slot i, run r]
            corr = blk - np.outer(p_pad, pad_m[g])
            for r in range(C):
                c = cmap_m[g, r]
                np.add.at(S, (c + ii) % C, corr[:, r])
                D[c] += corr[0, r]

    Ncnt = np.bincount(targets.reshape(-1).astype(np.int64),
                       minlength=C).astype(np.float64)
    return np.array(finish_loss(S, D, Ncnt, preds.shape[0] * HW),
                    dtype=np.float32)
